# revision 1
# baseline (speedup 1.0000x reference)
"""Trainium2 Bass kernel for nn_DecoderLayer_68212670595779.

Head-sharded attention + one 8-rank AllToAll, SPMD over 8 cores. See the
session memory for the full design. Measured: relative (absmax) error
5.419e-3 vs the fp32 reference; 407-410 us/layer (repeat-loop harness with
the collective stubbed), ~430 us single-shot with the real AllToAll.
"""
import sys

sys.path.insert(0, "/opt/trn_rl_repo")

import numpy as np
import ml_dtypes
from contextlib import ExitStack

import concourse.bass as bass
import concourse.mybir as mybir
import concourse.tile as tile
from concourse.vector_clock import ScopedClock
from concourse.bass_utils import run_bass_kernel_spmd

BF16 = ml_dtypes.bfloat16
FP32 = mybir.dt.float32
BF = mybir.dt.bfloat16
AF = mybir.ActivationFunctionType
ALU = mybir.AluOpType
AX = mybir.AxisListType

B, S, D, H, HD, FF, P = 2, 2048, 1024, 16, 64, 4096, 128
NCORES = 8
NU = [4 - t // 4 for t in range(16)]  # active q slots per kv chunk


# ---------------------------------------------------------------------------
# Workaround: this walrus build allows only ONE semaphore wait on a CTRL
# (Drain) instruction; TileContext's final drain carries one wait per busy
# proc. Split the waits across a chain of drains on the same engine.
def _patched_drain_and_barrier(self, tick_clock, wait_clock):
    nc = self.nc
    drain_inst = nc.sync.drain()
    wait_clock.add_sem_waits(
        drain_inst.ins, ScopedClock({None: tick_clock.global_clock})
    )
    si = drain_inst.ins.sync_info
    waits = list(si.on_wait) if si is not None else []
    if len(waits) > 1:
        si.on_wait = waits[:1]
        for w in waits[1:]:
            extra = nc.sync.drain()
            esi = extra.ins.sync_info
            if esi is None:
                extra.ins.sync_info = mybir.SyncInfo(on_wait=[w], on_update=[])
            else:
                esi.on_wait = [w]
    nc.all_engine_barrier()
    assert self.sems is not None
    popped = nc._tile_sem_poison_stack.pop()
    assert popped is self._sem_poison
    nc.clear_and_free_semaphores(list(self.sems.allocated().values()))
    nc.all_engine_barrier()


tile.TileContext._drain_and_barrier = _patched_drain_and_barrier


def _split_multi_waits(nc):
    """Walrus in this container supports a single sem wait per instruction.
    Move extra waits onto dedicated no-op instructions on the same engine,
    inserted immediately before (engine program order preserves semantics)."""
    n_split = 0
    for fn in nc.m.functions:
        for bb in fn.blocks:
            out = []
            for ins in bb.instructions:
                si = ins.sync_info
                waits = list(si.on_wait) if si is not None else []
                if len(waits) > 1:
                    si.on_wait = [waits[-1]]
                    for i, w in enumerate(waits[:-1]):
                        nop = mybir.InstNoOp(
                            name=f"{ins.name}-sw{i}",
                            engine=ins.engine,
                            bass_nofuse=True,
                            sync_info=mybir.SyncInfo(on_wait=[w], on_update=[]),
                        )
                        out.append(nop)
                        n_split += 1
                out.append(ins)
            bb.instructions[:] = out
    return n_split


def _core_plan(k):
    g = k % 4
    return k // 4, g, g  # batch, head-group, token-quarter


def _tri_mask():
    """{0,1}[kv 128, q 128] within-tile causal keep (kv <= q)."""
    a = np.arange(P)[:, None]
    qq = np.arange(P)[None, :]
    return (a <= qq).astype(np.float32).astype(BF16)


def _build_nc(reps=1, ablate=()):
    ablate = set(ablate)
    nc = bass.Bass()

    def din(name, shape, dt=BF):
        return nc.declare_dram_parameter(name, list(shape), dt, isOutput=False)

    xT_d = din("xT", (P, 8, S))
    mask_d = din("mask", (P, P))
    wq_d = din("wq", (P, 8, 256))
    wk_d = din("wk", (P, 8, 256))
    wv_d = din("wv", (P, 8, 256))
    wo_d = din("wo", (P, 8, D))
    w1_d = din("w1", (P, 8, FF))
    w2_d = din("w2", (P, 32, D))
    sel_d = din("sel", (P, 8), FP32)
    bqT_d = din("bqT", (P, 2), FP32)
    bkT_d = din("bkT", (P, 2), FP32)
    bvT_d = din("bvT", (P, 2), FP32)
    boT_d = din("boT", (P, 8), FP32)
    b1T_d = din("b1T", (P, 32), FP32)
    rows_d = din("rows", (3, D), FP32)  # b2 / gamma / beta
    eye_d = din("eye", (P, P), BF)
    out_d = nc.declare_dram_parameter("out", [512, D], FP32, isOutput=True)
    a2ain_d = nc.dram_tensor("a2ain", [2048, 512], BF)
    a2aout_d = nc.dram_tensor("a2aout", [2048, 512], BF)

    with ExitStack() as top:
        tc = top.enter_context(tile.TileContext(nc))

        const = top.enter_context(tc.tile_pool(name="const", bufs=1))
        persist = top.enter_context(tc.tile_pool(name="persist", bufs=1))

        # ---- constants ----
        ones_sb = const.tile([P, P], FP32, tag="ones")
        nc.vector.memset(ones_sb[:], 1.0)
        onesb_sb = const.tile([P, P], BF, tag="onesb")
        nc.vector.memset(onesb_sb[:], 1.0)
        eye_sb = const.tile([P, P], BF, tag="eye")
        nc.sync.dma_start(eye_sb[:], eye_d[:])
        bq_sb = const.tile([P, 2], FP32, tag="bq")
        nc.sync.dma_start(bq_sb[:], bqT_d[:])
        bk_sb = const.tile([P, 2], FP32, tag="bk")
        nc.sync.dma_start(bk_sb[:], bkT_d[:])
        bv_sb = const.tile([P, 2], FP32, tag="bv")
        nc.sync.dma_start(bv_sb[:], bvT_d[:])
        sel_sb = const.tile([P, 8], FP32, tag="sel")
        nc.sync.dma_start(sel_sb[:], sel_d[:])
        bo_sb = const.tile([P, 8], FP32, tag="bo")
        nc.sync.dma_start(bo_sb[:], boT_d[:])
        b1_sb = const.tile([P, 32], FP32, tag="b1")
        nc.sync.dma_start(b1_sb[:], b1T_d[:])
        # b2 at row 0, gamma at row 32, beta at row 64 (matmul rhs base part.)
        rows_sb = const.tile([P, D], FP32, tag="rows")
        nc.sync.dma_start(rows_sb[0:1, :], rows_d[0:1, :])
        nc.sync.dma_start(rows_sb[32:33, :], rows_d[1:2, :])
        nc.sync.dma_start(rows_sb[64:65, :], rows_d[2:3, :])

        attV = persist.tile([P, 2, S], BF, tag="attV")      # local heads
        attVf = persist.tile([P, 8, 512], BF, tag="attVf")  # post-A2A full
        attnTb = persist.tile([P, 8, 512], BF, tag="attnTb")
        if ablate:
            nc.vector.memset(attV[:], 0.25)
            nc.vector.memset(attVf[:], 0.25)
            nc.vector.memset(attnTb[:], 0.25)

        if reps > 1:
            # timing-only variant: repeat the whole body on-device so HW time
            # dominates host/tunnel dispatch overhead
            top.enter_context(tc.For_i(0, reps, 1))

        # =========================== phase 1 ===========================
        with ExitStack() as ph1:
            p1 = ph1.enter_context(tc.tile_pool(name="p1", bufs=1))
            KT = p1.tile([P, 2, S], BF, tag="KT")
            Vp = p1.tile([P, 16, 4 * 65], BF, tag="Vp")
            QT = p1.tile([P, 2, S], BF, tag="QT")

            for t in range(16):
                vv = Vp[:, t, :].rearrange("p (b j) -> p b j", j=65)
                nc.vector.memset(vv[:, :, 64:65], 1.0)

            with ExitStack() as ph1a:
                xp = ph1a.enter_context(tc.tile_pool(name="xp", bufs=1))
                wpool = ph1a.enter_context(tc.tile_pool(name="wqkv", bufs=2))
                pp_mm = ph1a.enter_context(
                    tc.tile_pool(name="ppmm1", bufs=3, space="PSUM")
                )

                xT_sb = xp.tile([P, 8, S], BF, tag="xT")
                nc.sync.dma_start(xT_sb[:], xT_d[:])
                _skip_proj = "compute" in ablate

                # ---- Q/K projections for local 4 heads: [256 dout, 2048] ----
                wq_sb = wpool.tile([P, 8, 256], BF, tag="w")
                nc.sync.dma_start(wq_sb[:], wq_d[:])
                wk_sb = wpool.tile([P, 8, 256], BF, tag="w")
                nc.sync.dma_start(wk_sb[:], wk_d[:])
                for dst, w_sb, b_sb in (
                    (QT, wq_sb, bq_sb),
                    (KT, wk_sb, bk_sb),
                ):
                    for m in range(2 if not _skip_proj else 0):
                        for ng in range(4):
                            ps = pp_mm.tile(
                                [P, 512], FP32, tag="mm",
                                name=f"qk_{m}_{ng}",
                            )
                            for kc in range(8):
                                nc.tensor.matmul(
                                    ps[:],
                                    lhsT=w_sb[:, kc, m * P : (m + 1) * P],
                                    rhs=xT_sb[:, kc, ng * 512 : (ng + 1) * 512],
                                    start=(kc == 0),
                                    stop=(kc == 7),
                                )
                            nc.vector.tensor_scalar_add(
                                dst[:, m, ng * 512 : (ng + 1) * 512],
                                ps[:],
                                b_sb[:, m : m + 1],
                            )

                # ---- V projection (local 4 heads, no bias) ----
                wv_sb = wpool.tile([P, 8, 256], BF, tag="w")
                nc.sync.dma_start(wv_sb[:], wv_d[:])
                for tt in range(16 if not _skip_proj else 0):
                    ps = pp_mm.tile([P, 512], FP32, tag="mm", name=f"v_{tt}")
                    for kc in range(8):
                        nc.tensor.matmul(
                            ps[:, :256],
                            lhsT=xT_sb[:, kc, tt * P : (tt + 1) * P],
                            rhs=wv_sb[:, kc, :],
                            start=(kc == 0),
                            stop=(kc == 7),
                        )
                    dst = Vp[:, tt, :].rearrange("p (b j) -> p b j", j=65)[
                        :, :, 0:64
                    ]
                    nc.vector.tensor_copy(
                        dst, ps[:, :256].rearrange("p (b j) -> p b j", j=64)
                    )

            # ---- attention: 2 local head pairs, contiguous causal ----
            with ExitStack() as ph1b:
                mp = ph1b.enter_context(tc.tile_pool(name="mp", bufs=1))
                ptp = ph1b.enter_context(tc.tile_pool(name="ptp", bufs=6))
                srec = ph1b.enter_context(tc.tile_pool(name="srec", bufs=2))
                stg = ph1b.enter_context(tc.tile_pool(name="stg", bufs=3))
                pp_s = ph1b.enter_context(
                    tc.tile_pool(name="pps", bufs=2, space="PSUM")
                )
                pp_o = ph1b.enter_context(
                    tc.tile_pool(name="ppo", bufs=4, space="PSUM")
                )

                mask_sb = mp.tile([P, P], BF, tag="mask")
                nc.sync.dma_start(mask_sb[:], mask_d[:])

                _n_hc = 0 if ("attn" in ablate or "compute" in ablate) else 2
                for hc in range(_n_hc):
                    for g4 in range(4):
                        o_pair = [
                            pp_o.tile(
                                [65, 512], FP32, tag="o", name=f"o_{hc}_{g4}_{i}"
                            )
                            for i in range(2)
                        ]
                        nt = 4 * g4 + 4
                        for t in range(nt):
                            r = t - 4 * g4
                            qoff = max(r, 0) * P
                            N = 512 - qoff
                            s_pair = pp_s.tile(
                                [P, 1024], FP32, tag="s", name=f"s_{hc}_{g4}_{t}"
                            )
                            for i, hp in enumerate((0, 64)):
                                nc.tensor.matmul(
                                    s_pair[:, i * 512 + qoff : (i + 1) * 512],
                                    lhsT=KT[hp : hp + 64, hc, t * P : (t + 1) * P],
                                    rhs=QT[
                                        hp : hp + 64,
                                        hc,
                                        g4 * 512 + qoff : (g4 + 1) * 512,
                                    ],
                                    start=True,
                                    stop=True,
                                )
                            pt = ptp.tile(
                                [P, 1024], BF, tag="pt", name=f"pt_{hc}_{g4}_{t}"
                            )
                            sv = s_pair[:].rearrange("p (h n) -> p h n", h=2)
                            pv = pt[:].rearrange("p (h n) -> p h n", h=2)
                            nc.scalar.activation(
                                pv[:, :, qoff:],
                                sv[:, :, qoff:],
                                AF.Exp,
                                scale=0.125,
                            )
                            if r >= 0:
                                nc.vector.tensor_tensor(
                                    pv[:, :, qoff : qoff + P],
                                    pv[:, :, qoff : qoff + P],
                                    mask_sb[:, None, :].to_broadcast([P, 2, P]),
                                    ALU.mult,
                                )
                            for i in range(2):
                                h = 2 * hc + i
                                nc.tensor.matmul(
                                    o_pair[i][:, qoff:],
                                    lhsT=Vp[:, t, h * 65 : (h + 1) * 65],
                                    rhs=pt[:, i * 512 + qoff : (i + 1) * 512],
                                    start=(t == 0),
                                    stop=(t == nt - 1),
                                )
                        # normalize into attV[local head, g4 block]
                        for i, hp in enumerate((0, 64)):
                            o_ps = o_pair[i]
                            rec = srec.tile(
                                [P, 512], FP32, tag="rec", name=f"rc_{hc}_{g4}_{i}"
                            )
                            nc.vector.reciprocal(rec[64:65, :], o_ps[64:65, :])
                            recb = srec.tile(
                                [P, 512], BF, tag="recb", name=f"rb_{hc}_{g4}_{i}"
                            )
                            nc.vector.tensor_copy(recb[64:65, :], rec[64:65, :])
                            rb = pp_s.tile(
                                [P, 1024], FP32, tag="s", name=f"rbp_{hc}_{g4}_{i}"
                            )
                            nc.tensor.matmul(
                                rb[0:64, :512],
                                lhsT=onesb_sb[64:65, 0:64],
                                rhs=recb[64:65, :],
                                start=True,
                                stop=True,
                            )
                            rbs = srec.tile(
                                [P, 512], FP32, tag="rbs", name=f"rs_{hc}_{g4}_{i}"
                            )
                            nc.vector.tensor_copy(rbs[0:64, :], rb[0:64, :512])
                            avs = attV[
                                hp : hp + 64, hc, g4 * 512 : (g4 + 1) * 512
                            ]
                            nc.vector.tensor_tensor(
                                avs, o_ps[0:64, :], rbs[0:64, :], ALU.mult
                            )
                            nc.vector.tensor_scalar_add(
                                avs, avs, bv_sb[hp : hp + 64, hc : hc + 1]
                            )

                # ---- stage (sel-zeroed) + AllToAll + recombine ----
                for j in range(8):
                    st = stg.tile([P, 2, 512], BF, tag="st", name=f"st_{j}")
                    nc.vector.tensor_scalar_mul(
                        st[:],
                        attV[:, :, (j % 4) * 512 : (j % 4 + 1) * 512],
                        sel_sb[:, j : j + 1],
                    )
                    nc.sync.dma_start(
                        a2ain_d[j * 256 : (j + 1) * 256, :].rearrange(
                            "(c p) q -> p c q", p=P
                        ),
                        st[:],
                    )
                if reps > 1:
                    # collectives cannot sit inside the timing repeat loop;
                    # substitute an equal-size local DMA (timing builds only)
                    nc.sync.dma_start(a2aout_d[:], a2ain_d[:])
                else:
                    nc.gpsimd.collective_compute(
                        "AllToAll",
                        ALU.bypass,
                        ins=[a2ain_d[:]],
                        outs=[a2aout_d[:]],
                        replica_groups=[[0, 1, 2, 3, 4, 5, 6, 7]],
                    )
                halfA = mp.tile([P, 8, 512], BF, tag="hA")
                nc.sync.dma_start(
                    halfA[:],
                    a2aout_d[0:1024, :].rearrange("(i p) q -> p i q", p=P),
                )
                halfB = mp.tile([P, 8, 512], BF, tag="hB")
                nc.sync.dma_start(
                    halfB[:],
                    a2aout_d[1024:2048, :].rearrange("(i p) q -> p i q", p=P),
                )
                nc.vector.tensor_tensor(attVf[:], halfA[:], halfB[:], ALU.add)

        # =========================== phase 2 ===========================
        with ExitStack() as ph2:
            p2 = ph2.enter_context(tc.tile_pool(name="p2", bufs=1))
            w1p = ph2.enter_context(tc.tile_pool(name="w1p", bufs=2))
            w2p = ph2.enter_context(tc.tile_pool(name="w2p", bufs=2))
            lnp = ph2.enter_context(tc.tile_pool(name="lnp", bufs=2))
            smal = ph2.enter_context(tc.tile_pool(name="smal", bufs=2))
            paux2 = ph2.enter_context(
                tc.tile_pool(name="paux2", bufs=2, space="PSUM")
            )
            pp_mm = ph2.enter_context(
                tc.tile_pool(name="ppmm2", bufs=3, space="PSUM")
            )

            # ---- Wo: attnTb[dout, q] (+bo) bf16 ----
            _skip_ffn = "ffn" in ablate or "compute" in ablate
            wo_sb = p2.tile([P, 8, D], BF, tag="wo")
            nc.sync.dma_start(wo_sb[:], wo_d[:])
            for m in range(8 if not _skip_ffn else 0):
                ps = pp_mm.tile([P, 512], FP32, tag="mm")
                for kc in range(8):
                    nc.tensor.matmul(
                        ps[:],
                        lhsT=wo_sb[:, kc, m * P : (m + 1) * P],
                        rhs=attVf[:, kc, :],
                        start=(kc == 0),
                        stop=(kc == 7),
                    )
                nc.vector.tensor_scalar_add(attnTb[:, m, :], ps[:], bo_sb[:, m : m + 1])

            # ---- broadcast rows b2/gamma/beta -> [128, 1024] fp32 ----
            b2b = p2.tile([P, D], FP32, tag="b2b")
            gb = p2.tile([P, D], FP32, tag="gb")
            bb = p2.tile([P, D], FP32, tag="bb")
            for rp, dst in ((0, b2b), (32, gb), (64, bb)):
                for hf in range(2):
                    psb = paux2.tile([P, 512], FP32, tag="aux")
                    nc.tensor.matmul(
                        psb[:],
                        lhsT=ones_sb[rp : rp + 1, :],
                        rhs=rows_sb[rp : rp + 1, hf * 512 : (hf + 1) * 512],
                        start=True,
                        stop=True,
                    )
                    nc.vector.tensor_copy(dst[:, hf * 512 : (hf + 1) * 512], psb[:])

            # ---- W1 + exact GELU (+b1): hT[f, q] bf16 ----
            hT = p2.tile([P, 32, 512], BF, tag="hT")
            for fg in range(8):
                w1_sb = w1p.tile([P, 8, 512], BF, tag="w1")
                nc.sync.dma_start(w1_sb[:], w1_d[:, :, fg * 512 : (fg + 1) * 512])
                for fs in range(4 if not _skip_ffn else 0):
                    f = fg * 4 + fs
                    ps = pp_mm.tile([P, 512], FP32, tag="mm")
                    for kc in range(8):
                        nc.tensor.matmul(
                            ps[:],
                            lhsT=w1_sb[:, kc, fs * P : (fs + 1) * P],
                            rhs=attnTb[:, kc, :],
                            start=(kc == 0),
                            stop=(kc == 7),
                        )
                    nc.scalar.activation(
                        hT[:, f, :], ps[:], AF.Gelu, bias=b1_sb[:, f : f + 1], scale=1.0
                    )

            # ---- transpose attnTb -> attn_sb[q, dout] fp32 (+b2 folded) ----
            attn_sb = p2.tile([P, 4, D], FP32, tag="attn")
            for m in range(8 if not _skip_ffn else 0):
                for t4 in range(4):
                    pst = paux2.tile([P, 512], BF, tag="auxb")
                    nc.tensor.transpose(
                        pst[:, 0:P], attnTb[:, m, t4 * P : (t4 + 1) * P], eye_sb[:]
                    )
                    nc.vector.tensor_tensor(
                        attn_sb[:, t4, m * P : (m + 1) * P],
                        pst[:, 0:P],
                        b2b[:, m * P : (m + 1) * P],
                        ALU.add,
                    )

            # ---- W2 + residual: y[q, dout] fp32 ----
            y_sb = p2.tile([P, 4, D], FP32, tag="y")
            for ng in range(2):
                # stream W2 in two half-contraction tiles for prefetch overlap
                w2_half = []
                for hf in range(2):
                    w2t = w2p.tile(
                        [P, 16, 512], BF, tag="w2", name=f"w2_{ng}_{hf}"
                    )
                    nc.sync.dma_start(
                        w2t[:],
                        w2_d[:, hf * 16 : (hf + 1) * 16, ng * 512 : (ng + 1) * 512],
                    )
                    w2_half.append(w2t)
                for t4 in range(4 if not _skip_ffn else 0):
                    ps = pp_mm.tile([P, 512], FP32, tag="mm")
                    for fc in range(32):
                        nc.tensor.matmul(
                            ps[:],
                            lhsT=hT[:, fc, t4 * P : (t4 + 1) * P],
                            rhs=w2_half[fc // 16][:, fc % 16, :],
                            start=(fc == 0),
                            stop=(fc == 31),
                        )
                    nc.vector.tensor_tensor(
                        y_sb[:, t4, ng * 512 : (ng + 1) * 512],
                        ps[:],
                        attn_sb[:, t4, ng * 512 : (ng + 1) * 512],
                        ALU.add,
                    )

            # ---- LayerNorm + out ----
            for t4 in range(4 if not _skip_ffn else 0):
                yv = y_sb[:, t4, :]
                s1 = smal.tile([P, 1], FP32, tag="s1")
                nc.vector.reduce_sum(s1[:], yv, axis=AX.X)
                sqo = lnp.tile([P, D], FP32, tag="sc")
                s2 = smal.tile([P, 1], FP32, tag="s2")
                nc.scalar.activation(sqo[:], yv, AF.Square, accum_out=s2[:])
                negmean = smal.tile([P, 1], FP32, tag="nm")
                nc.vector.tensor_scalar_mul(negmean[:], s1[:], -1.0 / D)
                mm2 = smal.tile([P, 1], FP32, tag="mm2")
                nc.vector.tensor_tensor(mm2[:], negmean[:], negmean[:], ALU.mult)
                bap = smal.tile([P, 1], FP32, tag="bap")
                nc.vector.tensor_scalar(bap[:], mm2[:], -1.0, 1e-6, ALU.mult, ALU.add)
                std = smal.tile([P, 1], FP32, tag="std")
                nc.scalar.activation(std[:], s2[:], AF.Sqrt, bias=bap[:], scale=1.0 / D)
                rstd = smal.tile([P, 1], FP32, tag="rstd")
                nc.vector.reciprocal(rstd[:], std[:])
                t1 = lnp.tile([P, D], FP32, tag="sc")
                nc.vector.tensor_scalar(
                    t1[:], yv, negmean[:], rstd[:], ALU.add, ALU.mult
                )
                nc.vector.tensor_tensor(t1[:], t1[:], gb[:], ALU.mult)
                nc.vector.tensor_tensor(t1[:], t1[:], bb[:], ALU.add)
                nc.sync.dma_start(out_d[t4 * P : (t4 + 1) * P, :], t1[:])

    _split_multi_waits(nc)
    return nc


_CACHE = {}


def _get_nc(reps=1, ablate=()):
    key = ("nc", reps, tuple(sorted(ablate)))
    if key not in _CACHE:
        _CACHE[key] = _build_nc(reps, ablate)
    return _CACHE[key]


def _prep_in_maps(x, mask, Wq, bq, Wk, bk, Wv, bv, Wo, bo, W1, b1, W2, b2, gamma, beta):
    x = np.asarray(x, np.float32)

    def chunkT(w, nch):
        return np.ascontiguousarray(
            np.asarray(w, np.float32).astype(BF16).reshape(nch, P, -1).transpose(1, 0, 2)
        )

    wo_h = chunkT(Wo, 8)
    w1_h = chunkT(W1, 8)
    w2_h = chunkT(W2, 32)
    Wq = np.asarray(Wq, np.float32)
    Wk = np.asarray(Wk, np.float32)
    Wv = np.asarray(Wv, np.float32)

    def bT(b, nch):
        return np.ascontiguousarray(np.asarray(b, np.float32).reshape(nch, P).T)

    bo_h = bT(bo, 8)
    b1_h = bT(b1, 32)
    rows_h = np.ascontiguousarray(
        np.stack(
            [
                np.asarray(b2, np.float32),
                np.asarray(gamma, np.float32),
                np.asarray(beta, np.float32),
            ]
        )
    )
    eye_h = np.eye(P, dtype=np.float32).astype(BF16)
    mask_h = _tri_mask()
    bq = np.asarray(bq, np.float32)
    bk = np.asarray(bk, np.float32)
    bv = np.asarray(bv, np.float32)

    in_maps = []
    plans = []
    for k in range(NCORES):
        b, g, c = _core_plan(k)
        xb = x[b]
        xT_h = np.ascontiguousarray(
            xb.T.astype(BF16).reshape(8, P, S).transpose(1, 0, 2)
        )
        hs = slice(g * 256, (g + 1) * 256)
        sel_h = np.zeros((P, 8), np.float32)
        sel_h[:, b * 4 : (b + 1) * 4] = 1.0
        in_maps.append(
            {
                "xT": xT_h,
                "mask": mask_h,
                "sel": sel_h,
                "wq": chunkT(Wq[:, hs], 8),
                "wk": chunkT(Wk[:, hs], 8),
                "wv": chunkT(Wv[:, hs], 8),
                "wo": wo_h,
                "w1": w1_h,
                "w2": w2_h,
                "bqT": bT(bq[hs], 2),
                "bkT": bT(bk[hs], 2),
                "bvT": bT(bv[hs], 2),
                "boT": bo_h,
                "b1T": b1_h,
                "rows": rows_h,
                "eye": eye_h,
            }
        )
        plans.append((b, c))
    return in_maps, plans


def kernel(**inputs):
    in_maps, plans = _prep_in_maps(**inputs)
    nc = _get_nc()
    res = run_bass_kernel_spmd(nc, in_maps, core_ids=list(range(NCORES)))
    out = np.zeros((B, S, D), np.float32)
    for k in range(NCORES):
        b, c = plans[k]
        out[b, c * 512 : (c + 1) * 512] = res.results[k]["out"]
    return out



# revision 61
# speedup vs baseline: 1.1788x; 1.1788x over previous
"""Trainium2 Bass kernel for nn_DecoderLayer_68212670595779.

v2 design (per core k: batch b=k//4, head-group g=k%4 [4 heads], quarter c=k%4):

Phase 1 — interleaved projections + attention, software-pipelined:
  for hc in 0,1 (head pair), g4 in 0..3 (512-token q block):
    project K/Q chunk (m=hc, ng=g4) [+ V tiles on hc=0] then run the
    attention block; attV(t-1) is emitted after scores/exp(t) so the PE
    never waits on the Activation engine's exp. bv is folded into V's
    projection (softmax rows sum to 1), denominators ride the 65th V row.
  After each head pair: one AllToAll over the 4 same-batch cores
  (replica groups [[0-3],[4-7]]) ships that pair's [4 quarters x 512]
  block; the hc=0 collective overlaps the hc=1 attention.

Phase 2 — Wo + FFN + LayerNorm for this core's 512 tokens, with weight
streaming prefetched during phase 1 (wo, first w1/w2 tiles).
"""
import sys

sys.path.insert(0, "/opt/trn_rl_repo")

import numpy as np
import ml_dtypes
from contextlib import ExitStack

import concourse.bass as bass
import concourse.mybir as mybir
import concourse.tile as tile
from concourse.vector_clock import ScopedClock
from concourse.bass_utils import run_bass_kernel_spmd

BF16 = ml_dtypes.bfloat16
FP32 = mybir.dt.float32
BF = mybir.dt.bfloat16
AF = mybir.ActivationFunctionType
ALU = mybir.AluOpType
AX = mybir.AxisListType

B, S, D, H, HD, FF, P = 2, 2048, 1024, 16, 64, 4096, 128
NCORES = 8


# ---------------------------------------------------------------------------
# Workaround: this walrus build allows only ONE semaphore wait on a CTRL
# (Drain) instruction; TileContext's final drain carries one wait per busy
# proc. Split the waits across a chain of drains on the same engine.
def _patched_drain_and_barrier(self, tick_clock, wait_clock):
    nc = self.nc
    drain_inst = nc.sync.drain()
    wait_clock.add_sem_waits(
        drain_inst.ins, ScopedClock({None: tick_clock.global_clock})
    )
    si = drain_inst.ins.sync_info
    waits = list(si.on_wait) if si is not None else []
    if len(waits) > 1:
        si.on_wait = waits[:1]
        for w in waits[1:]:
            extra = nc.sync.drain()
            esi = extra.ins.sync_info
            if esi is None:
                extra.ins.sync_info = mybir.SyncInfo(on_wait=[w], on_update=[])
            else:
                esi.on_wait = [w]
    nc.all_engine_barrier()
    assert self.sems is not None
    popped = nc._tile_sem_poison_stack.pop()
    assert popped is self._sem_poison
    nc.clear_and_free_semaphores(list(self.sems.allocated().values()))
    nc.all_engine_barrier()


tile.TileContext._drain_and_barrier = _patched_drain_and_barrier


def _split_multi_waits(nc):
    """Walrus in this container supports a single sem wait per instruction.
    Move extra waits onto dedicated no-op instructions on the same engine,
    inserted immediately before (engine program order preserves semantics)."""
    n_split = 0
    for fn in nc.m.functions:
        for bb in fn.blocks:
            out = []
            for ins in bb.instructions:
                si = ins.sync_info
                waits = list(si.on_wait) if si is not None else []
                if len(waits) > 1:
                    si.on_wait = [waits[-1]]
                    for i, w in enumerate(waits[:-1]):
                        nop = mybir.InstNoOp(
                            name=f"{ins.name}-sw{i}",
                            engine=ins.engine,
                            bass_nofuse=True,
                            sync_info=mybir.SyncInfo(on_wait=[w], on_update=[]),
                        )
                        out.append(nop)
                        n_split += 1
                out.append(ins)
            bb.instructions[:] = out
    return n_split


def _core_plan(k):
    g = k % 4
    return k // 4, g, g  # batch, head-group, token-quarter


def _tri_mask():
    """{0,1}[kv 128, q 128] within-tile causal keep (kv <= q)."""
    a = np.arange(P)[:, None]
    qq = np.arange(P)[None, :]
    return (a <= qq).astype(np.float32).astype(BF16)


def _build_nc(reps=1, ablate=(), stub_a2a=False, trivial_affine=True):
    """trivial_affine: gamma==1, beta==0, b2==0 (true for this problem's
    setup_inputs) — skips the LN scale/shift and the b2 residual add."""
    ablate = set(ablate)
    stub = (reps > 1) or stub_a2a
    nc = bass.Bass()

    def din(name, shape, dt=BF):
        return nc.declare_dram_parameter(name, list(shape), dt, isOutput=False)

    xT_d = din("xT", (P, 4, 8, 512))
    mask_d = din("mask", (P, P))
    wq_d = din("wq", (P, 8, 256))
    wk_d = din("wk", (P, 8, 256))
    wv_d = din("wv", (P, 8, 256))
    wo_d = din("wo", (P, 8, D))
    w1_d = din("w1", (P, 8, FF))
    w2_d = din("w2", (P, 32, D))
    bqT_d = din("bqT", (P, 2), FP32)
    bkT_d = din("bkT", (P, 2), FP32)
    bvb_d = din("bvb", (P, 256))
    boT_d = din("boT", (P, 8), FP32)
    b1T_d = din("b1T", (P, 32), FP32)
    b2b_d = din("b2b", (P, D))  # b2 row-broadcast, bf16
    gb_d = din("gb", (P, D), FP32)  # gamma row-broadcast
    bb_d = din("bb", (P, D), FP32)  # beta row-broadcast
    eye_d = din("eye", (P, P), BF)
    sel_d = din("sel", (P, 8), FP32)
    out_d = nc.declare_dram_parameter("out", [512, D], FP32, isOutput=True)
    a2a_in = [nc.dram_tensor(f"a2ain{h}", [1024, 512], BF) for h in range(2)]
    a2a_out = [nc.dram_tensor(f"a2aout{h}", [1024, 512], BF) for h in range(2)]

    _skip_attn = "attn" in ablate or "compute" in ablate
    _skip_ffn = "ffn" in ablate or "compute" in ablate

    with ExitStack() as top:
        tc = top.enter_context(tile.TileContext(nc))

        const = top.enter_context(tc.tile_pool(name="const", bufs=1))
        persist = top.enter_context(tc.tile_pool(name="persist", bufs=1))

        # ---- constants (outside the timing repeat loop) ----
        # small consts needed early; bulkier / later-phase consts are DMA'd
        # after the phase-1 critical loads (see below) to keep the queue clear
        onesb_sb = const.tile([P, P], BF, tag="onesb")
        nc.vector.memset(onesb_sb[:], 1.0)
        bq_sb = const.tile([P, 2], FP32, tag="bq")
        nc.sync.dma_start(bq_sb[:], bqT_d[:])
        bk_sb = const.tile([P, 2], FP32, tag="bk")
        nc.sync.dma_start(bk_sb[:], bkT_d[:])
        bvb_sb = const.tile([P, 256], BF, tag="bvb")
        nc.sync.dma_start(bvb_sb[:], bvb_d[:])
        mask_sb = const.tile([P, P], BF, tag="mask")
        nc.sync.dma_start(mask_sb[:], mask_d[:])
        sel_sb = const.tile([P, 8], FP32, tag="sel")
        nc.sync.dma_start(sel_sb[:], sel_d[:])
        eye_sb = const.tile([P, P], BF, tag="eye")
        bo_sb = const.tile([P, 8], FP32, tag="bo")
        b1_sb = const.tile([P, 32], FP32, tag="b1")
        b2b = const.tile([P, D], BF, tag="b2b")
        gb = const.tile([P, D], FP32, tag="gb")
        bb = const.tile([P, D], FP32, tag="bb")

        attV = persist.tile([P, 2, S], BF, tag="attV")
        attVf = persist.tile([P, 4, 2, 512], BF, tag="attVf")  # [p, src, hc, q]
        attnTb = persist.tile([P, 8, 512], BF, tag="attnTb")
        attnTbE = persist.tile([P, 8, 512], BF, tag="attnTbE")
        if ablate:
            nc.vector.memset(attV[:], 0.25)
            nc.vector.memset(attVf[:], 0.25)
            nc.vector.memset(attnTb[:], 0.25)
            nc.vector.memset(attnTbE[:], 0.0)

        if reps > 1:
            # timing-only variant: repeat the whole body on-device so HW time
            # dominates host/tunnel dispatch overhead
            top.enter_context(tc.For_i(0, reps, 1))

        # weight pools span both phases (prefetched during phase 1)
        wop = top.enter_context(tc.tile_pool(name="wop", bufs=1))
        w1p = top.enter_context(tc.tile_pool(name="w1p", bufs=2))
        w2p = top.enter_context(tc.tile_pool(name="w2p", bufs=2))

        # =========================== phase 1 ===========================
        with ExitStack() as ph1:
            p1 = ph1.enter_context(tc.tile_pool(name="p1", bufs=1))
            wqkv = ph1.enter_context(tc.tile_pool(name="wqkv", bufs=1))
            ptp = ph1.enter_context(tc.tile_pool(name="ptp", bufs=4))
            srec = ph1.enter_context(tc.tile_pool(name="srec", bufs=2))
            pp8 = ph1.enter_context(tc.tile_pool(name="pp8", bufs=4, space="PSUM"))
            pps = ph1.enter_context(tc.tile_pool(name="pps", bufs=2, space="PSUM"))

            xT_sb = p1.tile([P, 4, 8, 512], BF, tag="xT")
            KT = p1.tile([P, 2, S], BF, tag="KT")
            QT = p1.tile([P, 2, S], BF, tag="QT")
            Vp = p1.tile([P, 16, 4, 65], BF, tag="Vp")

            # ---- load order: K weights + first x chunk first ----
            wk_sb = wqkv.tile([P, 8, 256], BF, tag="wk")
            nc.sync.dma_start(wk_sb[:], wk_d[:])
            nc.sync.dma_start(xT_sb[:, 0], xT_d[:, 0])
            wq_sb = wqkv.tile([P, 8, 256], BF, tag="wq")
            nc.sync.dma_start(wq_sb[:], wq_d[:])
            wv_sb = wqkv.tile([P, 8, 256], BF, tag="wv")
            nc.sync.dma_start(wv_sb[:], wv_d[:])
            for ngl in range(1, 4):
                nc.sync.dma_start(xT_sb[:, ngl], xT_d[:, ngl])
            # deferred consts + phase-2 weight prefetch (DMA engine is idle
            # during attention)
            nc.sync.dma_start(eye_sb[:], eye_d[:])
            nc.sync.dma_start(bo_sb[:], boT_d[:])
            nc.sync.dma_start(b1_sb[:], b1T_d[:])
            if not trivial_affine:
                nc.sync.dma_start(b2b[:], b2b_d[:])
                nc.sync.dma_start(gb[:], gb_d[:])
                nc.sync.dma_start(bb[:], bb_d[:])
            wo_sb = wop.tile([P, 8, D], BF, tag="wo")
            nc.sync.dma_start(wo_sb[:], wo_d[:])
            w1_t0 = w1p.tile([P, 8, 512], BF, tag="w1", name="w1_0")
            nc.sync.dma_start(w1_t0[:], w1_d[:, :, 0:512])
            w2_t00 = w2p.tile([P, 8, 512], BF, tag="w2", name="w2_0_0")
            nc.sync.dma_start(w2_t00[:], w2_d[:, 0:8, 0:512])
            w2_t01 = w2p.tile([P, 8, 512], BF, tag="w2", name="w2_0_1")
            nc.sync.dma_start(w2_t01[:], w2_d[:, 8:16, 0:512])

            for tt in range(16):
                nc.vector.memset(Vp[:, tt, :, 64:65], 1.0)

            def emit_proj_one(which, m, ng):
                dst, w_sb, b_sb = {
                    "k": (KT, wk_sb, bk_sb),
                    "q": (QT, wq_sb, bq_sb),
                }[which]
                ps = pp8.tile([P, 512], FP32, tag="g", name=f"p{which}_{m}_{ng}")
                for kc in range(8):
                    nc.tensor.matmul(
                        ps[:],
                        lhsT=w_sb[:, kc, m * P : (m + 1) * P],
                        rhs=xT_sb[:, ng, kc, :],
                        start=(kc == 0),
                        stop=(kc == 7),
                    )
                nc.vector.tensor_scalar_add(
                    dst[:, m, ng * 512 : (ng + 1) * 512], ps[:], b_sb[:, m : m + 1]
                )

            def emit_proj_qk(m, ng):
                emit_proj_one("k", m, ng)
                emit_proj_one("q", m, ng)

            def emit_proj_v(tt):
                ps = pp8.tile([P, 512], FP32, tag="g", name=f"pv_{tt}")
                for kc in range(8):
                    nc.tensor.matmul(
                        ps[:, :256],
                        lhsT=xT_sb[:, tt // 4, kc, (tt % 4) * P : (tt % 4 + 1) * P],
                        rhs=wv_sb[:, kc, :],
                        start=(kc == 0),
                        stop=(kc == 7),
                    )
                nc.vector.tensor_tensor(
                    Vp[:, tt, :, 0:64],
                    ps[:, :256].rearrange("p (h j) -> p h j", j=64),
                    bvb_sb[:].rearrange("p (h j) -> p h j", j=64),
                    ALU.add,
                )

            pending_norm = []

            def flush_norm():
                while pending_norm:
                    pending_norm.pop(0)()

            def emit_attn_block(hc, g4, fillers=()):
                nt = 4 * g4 + 4
                fillers = list(fillers)
                # end-heavy placement: the ACT-vs-PE drift accumulates
                # through the block, so fillers go in the back half
                fpos = {nt - 1 - 2 * i: len(fillers) - 1 - i for i in range(len(fillers))}
                o_pair = [
                    pp8.tile([P, 512], FP32, tag="g", name=f"o_{hc}_{g4}_{i}")
                    for i in range(2)
                ]
                pend_av = None

                def emit_av(t, pt, qoff):
                    for i in range(2):
                        nc.tensor.matmul(
                            o_pair[i][0:65, qoff:],
                            lhsT=Vp[:, t, 2 * hc + i, :],
                            rhs=pt[:, i * 512 + qoff : (i + 1) * 512],
                            start=(t == 0),
                            stop=(t == nt - 1),
                        )

                for t in range(nt):
                    if t in fpos:
                        fillers[fpos[t]]()
                    r = t - 4 * g4
                    qoff = max(r, 0) * P
                    s_pair = pps.tile([P, 1024], FP32, tag="s", name=f"s_{hc}_{g4}_{t}")
                    for i, hp in enumerate((0, 64)):
                        nc.tensor.matmul(
                            s_pair[:, i * 512 + qoff : (i + 1) * 512],
                            lhsT=KT[hp : hp + 64, hc, t * P : (t + 1) * P],
                            rhs=QT[
                                hp : hp + 64, hc, g4 * 512 + qoff : (g4 + 1) * 512
                            ],
                            start=True,
                            stop=True,
                        )
                    pt = ptp.tile([P, 1024], BF, tag="pt", name=f"pt_{hc}_{g4}_{t}")
                    sv = s_pair[:].rearrange("p (h n) -> p h n", h=2)
                    pv = pt[:].rearrange("p (h n) -> p h n", h=2)
                    nc.scalar.activation(
                        pv[:, :, qoff:], sv[:, :, qoff:], AF.Exp, scale=0.125
                    )
                    if r >= 0:
                        nc.vector.tensor_tensor(
                            pv[:, :, qoff : qoff + P],
                            pv[:, :, qoff : qoff + P],
                            mask_sb[:, None, :].to_broadcast([P, 2, P]),
                            ALU.mult,
                        )
                    if pend_av is not None:
                        emit_av(*pend_av)
                    pend_av = (t, pt, qoff)
                emit_av(*pend_av)

                def norm():
                    rec = srec.tile([P, 1024], FP32, tag="rec", name=f"rf_{hc}_{g4}")
                    for i in range(2):
                        nc.vector.reciprocal(
                            rec[64:65, i * 512 : (i + 1) * 512], o_pair[i][64:65, :]
                        )
                    recb = srec.tile([P, 1024], BF, tag="recb", name=f"rc_{hc}_{g4}")
                    nc.vector.tensor_copy(recb[64:65, :], rec[64:65, :])
                    rb = pps.tile([P, 1024], FP32, tag="s", name=f"rbp_{hc}_{g4}")
                    for i in range(2):
                        nc.tensor.matmul(
                            rb[0:64, i * 512 : (i + 1) * 512],
                            lhsT=onesb_sb[64:65, 0:64],
                            rhs=recb[64:65, i * 512 : (i + 1) * 512],
                            start=True,
                            stop=True,
                        )
                    rbs = srec.tile([P, 1024], BF, tag="rbs", name=f"rs_{hc}_{g4}")
                    nc.vector.tensor_copy(rbs[0:64, :], rb[0:64, :])
                    for i, hp in enumerate((0, 64)):
                        nc.vector.tensor_tensor(
                            attV[hp : hp + 64, hc, g4 * 512 : (g4 + 1) * 512],
                            o_pair[i][0:64, :],
                            rbs[0:64, i * 512 : (i + 1) * 512],
                            ALU.mult,
                        )
                def emit_stage():
                    # stage this quarter for the head pair's 8-core AllToAll:
                    # destination j in {g4, g4+4}; sel zeroes the cross-batch
                    # copy (4-core replica groups are unsupported, so both
                    # batches ride one collective; the receiver adds halves)
                    for j in (g4, g4 + 4):
                        st = srec.tile(
                            [P, 512], BF, tag="st", name=f"st_{hc}_{g4}_{j}"
                        )
                        nc.vector.tensor_scalar_mul(
                            st[:],
                            attV[:, hc, g4 * 512 : (g4 + 1) * 512],
                            sel_sb[:, j : j + 1],
                        )
                        nc.sync.dma_start(a2a_in[hc][j * P : (j + 1) * P, :], st[:])
                    if stub:
                        # timing proxy for the collective hop, per quarter so
                        # it pipelines under the remaining attention blocks
                        for j in (g4, g4 + 4):
                            nc.sync.dma_start(
                                a2a_out[hc][j * P : (j + 1) * P, :],
                                a2a_in[hc][j * P : (j + 1) * P, :],
                            )
                        hA = srec.tile([P, 512], BF, tag="hA", name=f"hA_{hc}_{g4}")
                        nc.sync.dma_start(
                            hA[:], a2a_out[hc][g4 * P : (g4 + 1) * P, :]
                        )
                        hB = srec.tile([P, 512], BF, tag="hB", name=f"hB_{hc}_{g4}")
                        nc.sync.dma_start(
                            hB[:], a2a_out[hc][(g4 + 4) * P : (g4 + 5) * P, :]
                        )
                        nc.vector.tensor_tensor(
                            attVf[:, g4, hc, :], hA[:], hB[:], ALU.add
                        )

                def norm_and_stage():
                    norm()
                    emit_stage()

                pending_norm.append(norm_and_stage)

            def emit_a2a(hc):
                # quarters already staged to a2a_in by each norm(); exchange
                # among all 8 cores and add the two batch halves. In stub
                # (timing) builds the per-quarter proxy chain already ran.
                if not stub:
                    nc.gpsimd.collective_compute(
                        "AllToAll",
                        ALU.bypass,
                        ins=[a2a_in[hc][:]],
                        outs=[a2a_out[hc][:]],
                        replica_groups=[[0, 1, 2, 3, 4, 5, 6, 7]],
                    )
                    hA = p1.tile([P, 4, 512], BF, tag=f"hA{hc}")
                    nc.sync.dma_start(
                        hA[:], a2a_out[hc][0:512, :].rearrange("(s p) q -> p s q", p=P)
                    )
                    hB = p1.tile([P, 4, 512], BF, tag=f"hB{hc}")
                    nc.sync.dma_start(
                        hB[:],
                        a2a_out[hc][512:1024, :].rearrange("(s p) q -> p s q", p=P),
                    )
                    nc.vector.tensor_tensor(attVf[:, :, hc, :], hA[:], hB[:], ALU.add)

            # first half of Wo (even attn-dim chunks): depends only on the
            # FIRST AllToAll, so its groups double as late-phase-1 fillers
            def mk_woE(mq):
                def f():
                    ps = pp8.tile([P, 512], FP32, tag="g", name=f"woE_{mq}")
                    for j, kc in enumerate((0, 2, 4, 6)):
                        nc.tensor.matmul(
                            ps[:],
                            lhsT=wo_sb[:, kc, mq * P : (mq + 1) * P],
                            rhs=attVf[:, kc // 2, 0, :],
                            start=(j == 0),
                            stop=(j == 3),
                        )
                    nc.vector.tensor_copy(attnTbE[:, mq, :], ps[:])

                return f

            def mk_p(which, m, ng):
                return lambda: emit_proj_one(which, m, ng)

            # ---- phase-1 emission schedule ----
            # Interleave projection / WoE groups ("fillers") into the larger,
            # Activation-bound attention blocks so the PE never drains. hc=1
            # runs its largest q-block early and the smallest last, so the
            # final AllToAll fires with a minimal tail.
            # Each block takes ≤1 single-PSUM-group filler (two would cycle
            # the pp8 rotation onto a live accumulator before its deferred
            # normalize runs). Pattern between blocks: one covering unit,
            # then the deferred normalize, then the remaining units.
            if not _skip_attn:
                emit_proj_qk(0, 0)
                for tt in range(4):
                    emit_proj_v(tt)
                emit_attn_block(0, 0, [mk_p("k", 0, 1)])
                emit_proj_one("q", 0, 1)
                flush_norm()
                for tt in range(4, 8):
                    emit_proj_v(tt)
                emit_attn_block(0, 1, [mk_p("k", 0, 2)])
                emit_proj_one("q", 0, 2)
                flush_norm()
                for tt in range(8, 12):
                    emit_proj_v(tt)
                emit_attn_block(0, 2, [mk_p("k", 0, 3)])
                emit_proj_one("q", 0, 3)
                flush_norm()
                for tt in range(12, 16):
                    emit_proj_v(tt)
                emit_attn_block(0, 3, [mk_p("k", 1, 0)])
                emit_proj_one("q", 1, 0)
                flush_norm()
                emit_a2a(0)
                emit_proj_qk(1, 1)
                emit_attn_block(1, 1, [mk_p("k", 1, 2)])
                emit_proj_one("q", 1, 2)
                flush_norm()
                emit_attn_block(1, 2, [mk_p("k", 1, 3)])
                emit_proj_one("q", 1, 3)
                flush_norm()
                emit_attn_block(1, 3, [mk_woE(0)])
                mk_woE(1)()
                flush_norm()
                mk_woE(2)()
                mk_woE(3)()
                emit_attn_block(1, 0, [mk_woE(4)])
                mk_woE(5)()
                flush_norm()
                mk_woE(6)()
                mk_woE(7)()
                emit_a2a(1)

        # =========================== phase 2 ===========================
        with ExitStack() as ph2:
            p2 = ph2.enter_context(tc.tile_pool(name="p2", bufs=1))
            lnp = ph2.enter_context(tc.tile_pool(name="lnp", bufs=3))
            smal = ph2.enter_context(tc.tile_pool(name="smal", bufs=2))
            s1p = ph2.enter_context(tc.tile_pool(name="s1p", bufs=8))
            ppA = ph2.enter_context(tc.tile_pool(name="ppA", bufs=3, space="PSUM"))

            # ---- Wo: attnTb[dout, q] (+bo) bf16 ----
            # odd-kc half of Wo (the even half ran as phase-1 fillers);
            # kc descending: the s=0 chunk ships in the last AllToAll quarter,
            # so it's contracted last
            for mq in range(8 if not _skip_ffn else 0):
                ps = ppA.tile([P, 512], FP32, tag="mm", name=f"woO_{mq}")
                for j, kc in enumerate((7, 5, 3, 1)):
                    nc.tensor.matmul(
                        ps[:],
                        lhsT=wo_sb[:, kc, mq * P : (mq + 1) * P],
                        rhs=attVf[:, kc // 2, 1, :],
                        start=(j == 0),
                        stop=(j == 3),
                    )
                nc.vector.scalar_tensor_tensor(
                    attnTb[:, mq, :],
                    ps[:],
                    bo_sb[:, mq : mq + 1],
                    attnTbE[:, mq, :],
                    ALU.add,
                    ALU.add,
                )

            # ---- transpose attnTb -> attn_sb[q, dout] bf16 (+b2 folded) ----
            attn_sb = p2.tile([P, 4, D], BF, tag="attn")
            with ExitStack() as phT:
                ppT = phT.enter_context(
                    tc.tile_pool(name="ppT", bufs=2, space="PSUM")
                )
                for mq in range(8 if not _skip_ffn else 0):
                    pst = ppT.tile([P, 512], BF, tag="tp")
                    for t4 in range(4):
                        nc.tensor.transpose(
                            pst[:, t4 * P : (t4 + 1) * P],
                            attnTb[:, mq, t4 * P : (t4 + 1) * P],
                            eye_sb[:],
                        )
                    dst = attn_sb[:, :, mq * P : (mq + 1) * P]
                    src = pst[:].rearrange("p (t q) -> p t q", q=P)
                    if trivial_affine:
                        nc.vector.tensor_copy(dst, src)
                    else:
                        nc.vector.tensor_tensor(
                            dst,
                            src,
                            b2b[:, None, mq * P : (mq + 1) * P].to_broadcast(
                                [P, 4, P]
                            ),
                            ALU.add,
                        )

            # ---- W1 + exact GELU (+b1): hT[f, q] bf16 ----
            hT = p2.tile([P, 32, 512], BF, tag="hT")
            for fg in range(8):
                if fg == 0:
                    w1_sb = w1_t0
                else:
                    w1_sb = w1p.tile([P, 8, 512], BF, tag="w1", name=f"w1_{fg}")
                    nc.sync.dma_start(
                        w1_sb[:], w1_d[:, :, fg * 512 : (fg + 1) * 512]
                    )
                for fs in range(4 if not _skip_ffn else 0):
                    f = fg * 4 + fs
                    ps = ppA.tile([P, 512], FP32, tag="mm")
                    for kc in range(8):
                        nc.tensor.matmul(
                            ps[:],
                            lhsT=w1_sb[:, kc, fs * P : (fs + 1) * P],
                            rhs=attnTb[:, kc, :],
                            start=(kc == 0),
                            stop=(kc == 7),
                        )
                    nc.scalar.activation(
                        hT[:, f, :], ps[:], AF.Gelu, bias=b1_sb[:, f : f + 1], scale=1.0
                    )

            # ---- W2 + residual + LayerNorm, streamed by fc-chunk ----
            y_sb = p2.tile([P, 4, D], FP32, tag="y")
            s1h = [
                [s1p.tile([P, 1], FP32, tag="s1", name=f"s1_{t4}_{ng}") for ng in (0, 1)]
                for t4 in range(4)
            ]

            ln_state = {}

            def emit_ln_a(t4):
                # stats chain: cheap but latency-bound; runs under later W2 work
                yv = y_sb[:, t4, :]
                s1 = smal.tile([P, 1], FP32, tag="s1t", name=f"s1t_{t4}")
                nc.vector.tensor_tensor(s1[:], s1h[t4][0][:], s1h[t4][1][:], ALU.add)
                sqo = lnp.tile([P, D], FP32, tag="sc", name=f"sq_{t4}")
                s2 = smal.tile([P, 1], FP32, tag="s2", name=f"s2_{t4}")
                nc.scalar.activation(sqo[:], yv, AF.Square, accum_out=s2[:])
                negmean = smal.tile([P, 1], FP32, tag="nm", name=f"nm_{t4}")
                nc.vector.tensor_scalar_mul(negmean[:], s1[:], -1.0 / D)
                mm2 = smal.tile([P, 1], FP32, tag="mm2", name=f"mm2_{t4}")
                nc.vector.tensor_tensor(mm2[:], negmean[:], negmean[:], ALU.mult)
                bap = smal.tile([P, 1], FP32, tag="bap", name=f"bap_{t4}")
                nc.vector.tensor_scalar(
                    bap[:], mm2[:], -1.0, 1e-6, ALU.mult, ALU.add
                )
                std = smal.tile([P, 1], FP32, tag="std", name=f"std_{t4}")
                nc.scalar.activation(std[:], s2[:], AF.Sqrt, bias=bap[:], scale=1.0 / D)
                rstd = smal.tile([P, 1], FP32, tag="rstd", name=f"rstd_{t4}")
                nc.vector.reciprocal(rstd[:], std[:])
                ln_state[t4] = (negmean, rstd)

            def emit_ln_b(t4):
                yv = y_sb[:, t4, :]
                negmean, rstd = ln_state.pop(t4)
                t1 = lnp.tile([P, D], FP32, tag="sc", name=f"t1_{t4}")
                nc.vector.tensor_scalar(
                    t1[:], yv, negmean[:], rstd[:], ALU.add, ALU.mult
                )
                if not trivial_affine:
                    nc.vector.tensor_tensor(t1[:], t1[:], gb[:], ALU.mult)
                    nc.vector.tensor_tensor(t1[:], t1[:], bb[:], ALU.add)
                nc.sync.dma_start(out_d[t4 * P : (t4 + 1) * P, :], t1[:])

            with ExitStack() as phW2:
                ppW2 = phW2.enter_context(
                    tc.tile_pool(name="ppW2", bufs=4, space="PSUM")
                )
                for ng in range(2):
                    pss = [
                        ppW2.tile([P, 512], FP32, tag="w2m", name=f"w2ps_{ng}_{t4}")
                        for t4 in range(4)
                    ]
                    w2ts = []
                    for fcq in range(4):
                        if ng == 0 and fcq == 0:
                            w2t = w2_t00
                        elif ng == 0 and fcq == 1:
                            w2t = w2_t01
                        else:
                            w2t = w2p.tile(
                                [P, 8, 512], BF, tag="w2", name=f"w2_{ng}_{fcq}"
                            )
                            nc.sync.dma_start(
                                w2t[:],
                                w2_d[
                                    :,
                                    fcq * 8 : (fcq + 1) * 8,
                                    ng * 512 : (ng + 1) * 512,
                                ],
                            )
                        w2ts.append(w2t)
                        if fcq == 3:
                            break  # last chunk handled per-t4 below
                        for t4 in range(4 if not _skip_ffn else 0):
                            for fc8 in range(8):
                                fc = fcq * 8 + fc8
                                nc.tensor.matmul(
                                    pss[t4][:],
                                    lhsT=hT[:, fc, t4 * P : (t4 + 1) * P],
                                    rhs=w2t[:, fc8, :],
                                    start=(fc == 0),
                                    stop=False,
                                )
                    # last fc-chunk finished per-t4 so the residual add + LN
                    # of early t4s overlap the remaining matmuls
                    for t4 in range(4 if not _skip_ffn else 0):
                        for fc8 in range(8):
                            fc = 24 + fc8
                            nc.tensor.matmul(
                                pss[t4][:],
                                lhsT=hT[:, fc, t4 * P : (t4 + 1) * P],
                                rhs=w2ts[3][:, fc8, :],
                                start=False,
                                stop=(fc == 31),
                            )
                        nc.vector.scalar_tensor_tensor(
                            y_sb[:, t4, ng * 512 : (ng + 1) * 512],
                            pss[t4][:],
                            0.0,
                            attn_sb[:, t4, ng * 512 : (ng + 1) * 512],
                            ALU.add,
                            ALU.add,
                            accum_out=s1h[t4][ng][:],
                        )
                        if ng == 1:
                            emit_ln_a(t4)
                            if t4 > 0:
                                emit_ln_b(t4 - 1)
                            if t4 == 3:
                                emit_ln_b(3)
            if _skip_ffn:
                zt = lnp.tile([P, D], FP32, tag="sc")
                nc.vector.memset(zt[:], 0.0)
                for t4 in range(4):
                    nc.sync.dma_start(out_d[t4 * P : (t4 + 1) * P, :], zt[:])

    _split_multi_waits(nc)
    return nc


_CACHE = {}


def _get_nc(reps=1, ablate=(), trivial_affine=True):
    key = ("nc", reps, tuple(sorted(ablate)), trivial_affine)
    if key not in _CACHE:
        _CACHE[key] = _build_nc(reps, ablate, trivial_affine=trivial_affine)
    return _CACHE[key]


def _prep_in_maps(x, mask, Wq, bq, Wk, bk, Wv, bv, Wo, bo, W1, b1, W2, b2, gamma, beta):
    x = np.asarray(x, np.float32)

    def chunkT(w, nch):
        return np.ascontiguousarray(
            np.asarray(w, np.float32).astype(BF16).reshape(nch, P, -1).transpose(1, 0, 2)
        )

    wo_h = chunkT(Wo, 8)
    w1_h = chunkT(W1, 8)
    w2_h = chunkT(W2, 32)
    Wq = np.asarray(Wq, np.float32)
    Wk = np.asarray(Wk, np.float32)
    Wv = np.asarray(Wv, np.float32)

    def bT(b, nch):
        return np.ascontiguousarray(np.asarray(b, np.float32).reshape(nch, P).T)

    bo_h = bT(bo, 8)
    b1_h = bT(b1, 32)
    b2b_h = np.ascontiguousarray(
        np.broadcast_to(np.asarray(b2, np.float32).astype(BF16)[None, :], (P, D))
    )
    gb_h = np.ascontiguousarray(
        np.broadcast_to(np.asarray(gamma, np.float32)[None, :], (P, D))
    )
    bb_h = np.ascontiguousarray(
        np.broadcast_to(np.asarray(beta, np.float32)[None, :], (P, D))
    )
    eye_h = np.eye(P, dtype=np.float32).astype(BF16)
    mask_h = _tri_mask()
    bq = np.asarray(bq, np.float32)
    bk = np.asarray(bk, np.float32)
    bv = np.asarray(bv, np.float32)

    in_maps = []
    plans = []
    for k in range(NCORES):
        b, g, c = _core_plan(k)
        xb = x[b]
        # [S, D] -> [P, 4ng, 8kc, 512]
        xT_h = np.ascontiguousarray(
            xb.T.astype(BF16).reshape(8, P, 4, 512).transpose(1, 2, 0, 3)
        )
        hs = slice(g * 256, (g + 1) * 256)
        bvb_h = np.ascontiguousarray(
            np.broadcast_to(bv[hs].astype(BF16)[None, :], (P, 256))
        )
        sel_h = np.zeros((P, 8), np.float32)
        sel_h[:, b * 4 : (b + 1) * 4] = 1.0
        in_maps.append(
            {
                "sel": sel_h,
                "xT": xT_h,
                "mask": mask_h,
                "wq": chunkT(Wq[:, hs], 8),
                "wk": chunkT(Wk[:, hs], 8),
                "wv": chunkT(Wv[:, hs], 8),
                "wo": wo_h,
                "w1": w1_h,
                "w2": w2_h,
                "bqT": bT(bq[hs], 2),
                "bkT": bT(bk[hs], 2),
                "bvb": bvb_h,
                "boT": bo_h,
                "b1T": b1_h,
                "b2b": b2b_h,
                "gb": gb_h,
                "bb": bb_h,
                "eye": eye_h,
            }
        )
        plans.append((b, c))
    return in_maps, plans


def kernel(**inputs):
    in_maps, plans = _prep_in_maps(**inputs)
    triv = bool(
        np.all(np.asarray(inputs["gamma"], np.float32) == 1.0)
        and np.all(np.asarray(inputs["beta"], np.float32) == 0.0)
        and np.all(np.asarray(inputs["b2"], np.float32) == 0.0)
    )
    nc = _get_nc(trivial_affine=triv)
    res = run_bass_kernel_spmd(nc, in_maps, core_ids=list(range(NCORES)))
    out = np.zeros((B, S, D), np.float32)
    for k in range(NCORES):
        b, c = plans[k]
        out[b, c * 512 : (c + 1) * 512] = res.results[k]["out"]
    return out
